# revision 38
# baseline (speedup 1.0000x reference)
"""Trainium2 Bass kernel for the batch ConsistencyLoss (masked pairwise KL).

Math (reference):
    emb = x / ||x||;  sim = emb @ emb.T;  mask = (sim > 0.8) & ~eye
    L = log_softmax(routing);  P = exp(L);  ne[j] = sum_k P[j,k] L[j,k]
    kl[i,j] = ne[j] - (L @ P.T)[i,j]
    loss = sum(mask * kl) / count(mask)

Device algorithm (per core, row strip S of 1024 rows):
  * Rows are normalized to ||u|| = 16 in the natural [row, H] layout
    (ACT Square+accum for ||x||^2, DVE multiply), quantized to fp8e4 at
    the PSUM->SBUF copy-out of the PE transpose.  The similarity test is
    then a compare against the CONSTANT 0.8*16*16 = 204.8.
  * sim is computed TRANSPOSED, one 128-row block of j at a time:
        simT[j, i] = q(u_j) . q(u_i),   j in block jt, i in strip S
    with fp8 DoubleRow matmuls (contraction 256/instruction, stationary
    = embt block, moving = strip columns).  maskT = simT > 204.8.
  * Masked-KL sum, fully on the transposed side:
        V[k, i]  = sum_j W18[j, k] * maskT[j, i]   (PSUM-accumulated
                   over ALL jt blocks; W18 = [-P | ne | 1], 18 cols)
        masked_sum = sum_{k,i} LTpad[k, i] * V[k, i]
        count      = sum_i V[17, i]
    with LTpad = [L_S^T ; 1 ; 0] (18 x S).  No U staging or transposes.
  * The jt sim block is emitted right after embedding tile jt is
    transposed, so DMA / normalize / transpose / sim / V form one
    software-pipelined loop that keeps the PE continuously busy.
  * Diagonal pairs have kl == 0 exactly, so they stay in the mask and
    the host subtracts B from the pair count.
"""

import numpy as np

import concourse.bacc as bacc
import concourse.tile as tile
from concourse import mybir
from concourse.bass_utils import run_bass_kernel_spmd
from concourse.masks import make_identity

B, E, H = 8192, 16, 1024
NCORES = 8
STRIP = B // NCORES  # 1024 rows per core
MT = STRIP // 128    # 8 row chunks per strip
KT = H // 128        # 8 contraction tiles
KT2 = KT // 2        # 4 DoubleRow contraction pairs
BT = B // 128        # 64 batch tiles (also the jt sim blocks)
W = E + 2            # [-P | ne | 1] stationary width
SIM_THRESHOLD = 0.8
SCALE = 16.0         # rows normalized to this L2 norm before fp8 quant
THRESH = SIM_THRESHOLD * SCALE * SCALE  # 204.8 in raw-dot units
WEIGHT = 1.0
F32 = mybir.dt.float32
BF16 = mybir.dt.bfloat16
F8 = mybir.dt.float8e4
AX = mybir.AxisListType.X
OP = mybir.AluOpType
AF = mybir.ActivationFunctionType
DR = mybir.MatmulPerfMode.DoubleRow


def _kernel(tc, emb, emb_s, rp, rp_s, out_dram, reps=1, loop_iters=None,
            phases="ABCD"):
    nc = tc.nc
    with tc.tile_pool(name="persist", bufs=1) as persist:
        embt = persist.tile([128, KT, B], F8)          # q(u)^T [h%128,kt,b]
        stript = persist.tile([128, KT, STRIP], F8)    # strip columns
        W18 = persist.tile([128, BT, W], BF16)         # [-P | ne | 1]
        LTpad = persist.tile([W, STRIP], F32)          # [L^T ; 1 ; 0]
        identf = persist.tile([128, 128], F32)
        identb = persist.tile([128, 128], BF16)
        ones18 = persist.tile([W, 1], F32)
        make_identity(nc, identf)
        make_identity(nc, identb)
        nc.vector.memset(ones18, 1.0)
        if "B" not in phases and "C" in phases:
            # timing variants only: C reads embt/stript without B writing
            for kt in range(KT):
                nc.gpsimd.memset(embt[:, kt, :], 0.5)
                nc.gpsimd.memset(stript[:, kt, :], 0.5)
        if "A" not in phases and "C" in phases:
            nc.gpsimd.memset(W18, 0.01)
            nc.gpsimd.memset(LTpad, 0.01)

        args = (tc, nc, emb, emb_s, rp, rp_s, out_dram, embt, stript,
                W18, LTpad, identf, identb, ones18)
        if loop_iters is not None:
            with tc.For_i(0, loop_iters, 1):
                _phases(*args, "", phases)
            return
        for rep in range(reps):
            _phases(*args, f"r{rep}_" if reps > 1 else "", phases)


def _phases(tc, nc, emb, emb_s, rp, rp_s, out_dram, embt, stript,
            W18, LTpad, identf, identb, ones18, r, which="ABCD"):
    # ---- Phase A: softmax stats (full batch -P/ne/1; strip L^T) ----
    # All Exp ops batch under one ACT table; the 72 Ln calls collapse into
    # ONE Ln over the collected sums (ACT table loads: ~2 instead of ~99).
    TT = BT + MT
    if "A" in which:
     with tc.tile_pool(name=f"{r}smx", bufs=1) as smx, \
          tc.tile_pool(name=f"{r}ltp", bufs=2, space="PSUM") as ltps:
        rp_sb = smx.tile([128, BT, E], F32, tag="rp_sb")
        rps_sb = smx.tile([128, MT, E], F32, tag="rps_sb")
        nc.sync.dma_start(
            out=rp_sb, in_=rp.rearrange("(bt p) e -> p bt e", p=128))
        nc.sync.dma_start(
            out=rps_sb, in_=rp_s.rearrange("(mt p) e -> p mt e", p=128))
        # Logits are N(0,1) so exp cannot overflow: skip the max-shift and
        # batch everything.  L = x - log(sum exp x);  ne = (sum e*x)/s - logs.
        e_all = smx.tile([128, TT, E], F32, tag="e_all")
        s_all = smx.tile([128, TT], F32, tag="s_all")
        logs_all = smx.tile([128, TT], F32, tag="logs_all")
        rs_all = smx.tile([128, TT], F32, tag="rs_all")
        nc.scalar.activation(out=e_all[:, 0:BT, :], in_=rp_sb, func=AF.Exp)
        nc.scalar.activation(out=e_all[:, BT:TT, :], in_=rps_sb, func=AF.Exp)
        nc.vector.reduce_sum(out=s_all, in_=e_all, axis=AX)
        nc.scalar.activation(out=logs_all, in_=s_all, func=AF.Ln)
        nc.vector.reciprocal(out=rs_all, in_=s_all)
        nc.vector.memset(W18[:, :, E + 1], 1.0)
        # ne_all = (sum_k e*x)*rs - logs, all [128, TT] batched
        prodel = smx.tile([128, TT, E], F32, tag="prodel")
        nc.vector.tensor_tensor(out=prodel[:, 0:BT, :], in0=e_all[:, 0:BT, :],
                                in1=rp_sb, op=OP.mult)
        nc.vector.tensor_tensor(out=prodel[:, BT:TT, :],
                                in0=e_all[:, BT:TT, :], in1=rps_sb,
                                op=OP.mult)
        epx = smx.tile([128, TT], F32, tag="epx")
        nc.vector.reduce_sum(out=epx, in_=prodel, axis=AX)
        ne_all = smx.tile([128, TT], F32, tag="ne_all")
        nc.vector.tensor_tensor(out=ne_all, in0=epx, in1=rs_all, op=OP.mult)
        nc.vector.tensor_tensor(out=ne_all, in0=ne_all, in1=logs_all,
                                op=OP.subtract)
        with nc.allow_low_precision(reason="ne copy to bf16 table"):
            nc.vector.tensor_copy(
                out=W18[:, :, E:E + 1],
                in_=ne_all.rearrange("p (t o) -> p t o", o=1)[:, 0:BT, :])
        for bt in range(BT):
            nc.vector.tensor_scalar(W18[:, bt, 0:E], e_all[:, bt, :],
                                    rs_all[:, bt:bt + 1], -1.0,
                                    op0=OP.mult, op1=OP.mult)
        # LTpad rows 0..15 = L^T, row 16 = 1 (via transpose of [L | 1]),
        # row 17 = 0 (whole-tile memset; partition slices must start at 0)
        nc.vector.memset(LTpad, 0.0)
        for ms in range(MT):
            t = BT + ms
            Lm = smx.tile([128, E + 1], F32, tag="Lm", bufs=3)
            nc.vector.memset(Lm[:, E:E + 1], 1.0)
            nc.vector.tensor_scalar(Lm[:, 0:E], rps_sb[:, ms, :],
                                    logs_all[:, t:t + 1], None,
                                    op0=OP.subtract)
            lt = ltps.tile([E + 1, 128], F32, tag="lt")
            nc.tensor.matmul(out=lt, lhsT=Lm, rhs=identf,
                             start=True, stop=True)
            nc.scalar.copy(out=LTpad[0:E + 1, ms * 128:(ms + 1) * 128],
                           in_=lt)

    # ---- Phase B+C: merged pipeline ----
    #   prep(bt):  DMA -> norm^2 (ACT) -> 16/||x|| (DVE) -> normalize (DVE)
    #              -> PE transpose -> fp8 copy-out (ACT/DVE alternating)
    #   simblk(jt): 8 DoubleRow matmuls (2 column halves) -> maskT (DVE)
    #              -> V matmul (accumulated over all jt)
    # simblk(jt) is emitted two prep steps behind, V one jt behind, so no
    # in-order engine stream ever waits on same-step work.
    do_b = "B" in which
    do_c = "C" in which
    if do_b or do_c:
     with tc.tile_pool(name=f"{r}vps", bufs=1, space="PSUM") as vps:
      V = vps.tile([W, STRIP], F32, name="V") if do_c else None
      with tc.tile_pool(name=f"{r}embp", bufs=3) as ep, \
           tc.tile_pool(name=f"{r}trps", bufs=2, space="PSUM") as trps, \
           tc.tile_pool(name=f"{r}simps", bufs=3, space="PSUM") as sps, \
           tc.tile_pool(name=f"{r}mkp", bufs=6) as mkp:
        tpend = []   # pending transpose copy-outs  (tp, dst, bt)
        vpend = []   # pending V matmuls            (jt, [msk0, msk1])

        def drain_tp():
            tp_, dst_, bt_ = tpend.pop(0)
            eng = (nc.scalar.copy if bt_ % 2 == 0 else nc.vector.tensor_copy)
            eng(out=dst_[:, :, bt_ * 128:(bt_ + 1) * 128],
                in_=tp_.rearrange("p (k c) -> p k c", k=KT))

        def prep(src_ap, dst_tile, bt):
            x = ep.tile([128, H], F32, tag="ex", bufs=4)
            # split loads across the SP hwdge queue and gpsimd's swdge queue
            # (gpsimd is otherwise idle, so its triggers never stall)
            dma_eng = nc.sync if bt % 2 == 0 else nc.gpsimd
            dma_eng.dma_start(out=x, in_=src_ap[bt * 128:(bt + 1) * 128, :])
            scr = ep.tile([128, H], BF16, tag="sqscr", bufs=2)
            ss = ep.tile([128, 1], F32, tag="ss", bufs=3)
            nc.scalar.activation(out=scr, in_=x, func=AF.Square,
                                 accum_out=ss)
            # n16 = ||x||/16;  rs = 16/||x||
            n16 = ep.tile([128, 1], F32, tag="n16", bufs=3)
            nc.scalar.activation(out=n16, in_=ss, func=AF.Sqrt, bias=0.0,
                                 scale=1.0 / (SCALE * SCALE))
            rs = ep.tile([128, 1], F32, tag="rs", bufs=3)
            nc.vector.reciprocal(out=rs, in_=n16)
            xq = ep.tile([128, H], BF16, tag="xq", bufs=3)
            nc.vector.tensor_scalar(xq, x, rs, None, op0=OP.mult)
            if tpend:
                drain_tp()
            tp = trps.tile([128, H], BF16, tag="tr")
            for kt in range(KT):
                nc.tensor.transpose(tp[:, kt * 128:(kt + 1) * 128],
                                    xq[:, kt * 128:(kt + 1) * 128],
                                    identb)
            tpend.append((tp, dst_tile, bt))

        def drain_v(stop):
            jt_, msks_ = vpend.pop(0)
            for hh in range(2):
                nc.tensor.matmul(out=V[:, hh * 512:(hh + 1) * 512],
                                 lhsT=W18[:, jt_, :], rhs=msks_[hh],
                                 start=(jt_ == 0), stop=stop)

        def simblk(jt):
            # k2 outer / hh inner: consecutive matmuls share the stationary
            # operand, so each LDWEIGHTS hides under 2x512 moving cycles
            simT0 = sps.tile([128, 512], F32, tag="simT0", bufs=2)
            simT1 = sps.tile([128, 512], F32, tag="simT1", bufs=2)
            halves = (simT0, simT1)
            for k2 in range(KT2):
                for hh in range(2):
                    nc.tensor.matmul(
                        out=halves[hh],
                        lhsT=embt[:, 2 * k2:2 * k2 + 2,
                                  jt * 128:(jt + 1) * 128],
                        rhs=stript[:, 2 * k2:2 * k2 + 2,
                                   hh * 512:(hh + 1) * 512],
                        start=(k2 == 0), stop=(k2 == KT2 - 1),
                        perf_mode=DR)
            msks = []
            for hh in range(2):
                msk = mkp.tile([128, 512], BF16, tag="mask")
                nc.vector.tensor_scalar(msk, halves[hh], THRESH, None,
                                        op0=OP.is_gt)
                msks.append(msk)
            if vpend:
                drain_v(False)
            vpend.append((jt, msks))

        if do_b:
            for ms in range(MT):           # strip prologue
                prep(emb_s, stript, ms)
        CLAG = 2
        for bt in range(BT):
            if do_b:
                prep(emb, embt, bt)
            if do_c and bt >= CLAG:
                simblk(bt - CLAG)
        for jt in range(BT - CLAG, BT) if do_c else []:
            simblk(jt)
        while tpend:
            drain_tp()
        if do_c:
            drain_v(True)

      # ---- readout: masked_sum and count from V (pipeline pools closed,
      # so the PSUM bank for the final matmul is free) ----
      if do_c:
            with tc.tile_pool(name=f"{r}fin", bufs=1) as fin, \
                 tc.tile_pool(name=f"{r}fps", bufs=1, space="PSUM") as fps:
                Vs = fin.tile([W, STRIP], F32)
                nc.scalar.copy(out=Vs, in_=V)
                scr = fin.tile([W, STRIP], F32)
                nc.vector.tensor_tensor(out=scr, in0=Vs, in1=LTpad,
                                        op=OP.mult)
                # accs col0 = rowsum(LTpad*V) (-> masked_sum), col1 =
                # rowsum(V); select row 17 of col1 (count) by multiplying
                # with [1 | e17] built from the identity's column 17.
                accs = fin.tile([W, 2], F32)
                nc.vector.reduce_sum(out=accs[:, 0:1], in_=scr, axis=AX)
                nc.vector.reduce_sum(out=accs[:, 1:2], in_=Vs, axis=AX)
                sel = fin.tile([W, 2], F32)
                nc.vector.tensor_copy(out=sel[:, 0:1], in_=ones18)
                nc.vector.tensor_copy(out=sel[:, 1:2],
                                      in_=identf[0:W, W - 1:W])
                msel = fin.tile([W, 2], F32)
                nc.vector.tensor_tensor(out=msel, in0=accs, in1=sel,
                                        op=OP.mult)
                res = fps.tile([1, 2], F32)
                nc.tensor.matmul(out=res, lhsT=ones18, rhs=msel,
                                 start=True, stop=True)
                out_sb = fin.tile([1, 2], F32)
                nc.scalar.copy(out=out_sb, in_=res)
                nc.sync.dma_start(out=out_dram, in_=out_sb)


def build_bass(reps=1, loop_iters=None, phases="ABCD"):
    nc = bacc.Bacc("TRN2", target_bir_lowering=False, debug=False)
    emb = nc.dram_tensor("emb", [B, H], F32, kind="ExternalInput").ap()
    emb_s = nc.dram_tensor("emb_strip", [STRIP, H], F32,
                           kind="ExternalInput").ap()
    rp = nc.dram_tensor("rp", [B, E], F32, kind="ExternalInput").ap()
    rp_s = nc.dram_tensor("rp_strip", [STRIP, E], F32,
                          kind="ExternalInput").ap()
    out = nc.dram_tensor("out", [1, 2], F32, kind="ExternalOutput").ap()
    with tile.TileContext(nc) as tc:
        _kernel(tc, emb, emb_s, rp, rp_s, out, reps=reps,
                loop_iters=loop_iters, phases=phases)
    nc.compile()
    return nc


_NC_CACHE = None


def make_in_maps(rp, emb):
    in_maps = []
    for d in range(NCORES):
        in_maps.append({
            "emb": emb,
            "emb_strip": np.ascontiguousarray(emb[d * STRIP:(d + 1) * STRIP]),
            "rp": rp,
            "rp_strip": np.ascontiguousarray(rp[d * STRIP:(d + 1) * STRIP]),
        })
    return in_maps


def kernel(routing_probs: np.ndarray, input_embeddings: np.ndarray,
           **_unused) -> np.ndarray:
    global _NC_CACHE
    if _NC_CACHE is None:
        _NC_CACHE = build_bass()
    nc = _NC_CACHE
    rp = np.ascontiguousarray(routing_probs, dtype=np.float32)
    emb = np.ascontiguousarray(input_embeddings, dtype=np.float32)
    in_maps = make_in_maps(rp, emb)
    res = run_bass_kernel_spmd(nc, in_maps, core_ids=list(range(NCORES)))
    vals = np.array([r["out"].reshape(2) for r in res.results],
                    dtype=np.float64)
    total = vals[:, 0].sum()
    cnt = vals[:, 1].sum() - B  # drop the diagonal pairs (kl there is 0)
    if cnt > 0:
        loss = np.float32(total) / np.float32(max(cnt, 1.0))
    else:
        loss = 0.0
    return np.array(WEIGHT * loss, dtype=np.float32)


# revision 39
# speedup vs baseline: 1.0456x; 1.0456x over previous
"""Trainium2 Bass kernel for the batch ConsistencyLoss (masked pairwise KL).

Math (reference):
    emb = x / ||x||;  sim = emb @ emb.T;  mask = (sim > 0.8) & ~eye
    L = log_softmax(routing);  P = exp(L);  ne[j] = sum_k P[j,k] L[j,k]
    kl[i,j] = ne[j] - (L @ P.T)[i,j]
    loss = sum(mask * kl) / count(mask)

Device algorithm (per core, row strip S of 1024 rows):
  * Rows are normalized to ||u|| = 16 in the natural [row, H] layout
    (ACT Square+accum for ||x||^2, DVE multiply), quantized to fp8e4 at
    the PSUM->SBUF copy-out of the PE transpose.  The similarity test is
    then a compare against the CONSTANT 0.8*16*16 = 204.8.
  * sim is computed TRANSPOSED, one 128-row block of j at a time:
        simT[j, i] = q(u_j) . q(u_i),   j in block jt, i in strip S
    with fp8 DoubleRow matmuls (contraction 256/instruction, stationary
    = embt block, moving = strip columns).  maskT = simT > 204.8.
  * Masked-KL sum, fully on the transposed side:
        V[k, i]  = sum_j W18[j, k] * maskT[j, i]   (PSUM-accumulated
                   over ALL jt blocks; W18 = [-P | ne | 1], 18 cols)
        masked_sum = sum_{k,i} LTpad[k, i] * V[k, i]
        count      = sum_i V[17, i]
    with LTpad = [L_S^T ; 1 ; 0] (18 x S).  No U staging or transposes.
  * The jt sim block is emitted right after embedding tile jt is
    transposed, so DMA / normalize / transpose / sim / V form one
    software-pipelined loop that keeps the PE continuously busy.
  * Diagonal pairs have kl == 0 exactly, so they stay in the mask and
    the host subtracts B from the pair count.
"""

import numpy as np

import concourse.bacc as bacc
import concourse.tile as tile
from concourse import mybir
from concourse.bass_utils import run_bass_kernel_spmd
from concourse.masks import make_identity

B, E, H = 8192, 16, 1024
NCORES = 8
STRIP = B // NCORES  # 1024 rows per core
MT = STRIP // 128    # 8 row chunks per strip
KT = H // 128        # 8 contraction tiles
KT2 = KT // 2        # 4 DoubleRow contraction pairs
BT = B // 128        # 64 batch tiles (also the jt sim blocks)
W = E + 2            # [-P | ne | 1] stationary width
SIM_THRESHOLD = 0.8
SCALE = 16.0         # rows normalized to this L2 norm before fp8 quant
THRESH = SIM_THRESHOLD * SCALE * SCALE  # 204.8 in raw-dot units
WEIGHT = 1.0
F32 = mybir.dt.float32
BF16 = mybir.dt.bfloat16
F8 = mybir.dt.float8e4
AX = mybir.AxisListType.X
OP = mybir.AluOpType
AF = mybir.ActivationFunctionType
DR = mybir.MatmulPerfMode.DoubleRow


def _kernel(tc, emb, emb_s, rp, rp_s, out_dram, reps=1, loop_iters=None,
            phases="ABCD"):
    nc = tc.nc
    with tc.tile_pool(name="persist", bufs=1) as persist:
        embt = persist.tile([128, KT, B], F8)          # q(u)^T [h%128,kt,b]
        stript = persist.tile([128, KT, STRIP], F8)    # strip columns
        W18 = persist.tile([128, BT, W], BF16)         # [-P | ne | 1]
        LTpad = persist.tile([W, STRIP], F32)          # [L^T ; 1 ; 0]
        identf = persist.tile([128, 128], F32)
        identb = persist.tile([128, 128], BF16)
        ones18 = persist.tile([W, 1], F32)
        make_identity(nc, identf)
        make_identity(nc, identb)
        nc.vector.memset(ones18, 1.0)
        if "B" not in phases and "C" in phases:
            # timing variants only: C reads embt/stript without B writing
            for kt in range(KT):
                nc.gpsimd.memset(embt[:, kt, :], 0.5)
                nc.gpsimd.memset(stript[:, kt, :], 0.5)
        if "A" not in phases and "C" in phases:
            nc.gpsimd.memset(W18, 0.01)
            nc.gpsimd.memset(LTpad, 0.01)

        args = (tc, nc, emb, emb_s, rp, rp_s, out_dram, embt, stript,
                W18, LTpad, identf, identb, ones18)
        if loop_iters is not None:
            with tc.For_i(0, loop_iters, 1):
                _phases(*args, "", phases)
            return
        for rep in range(reps):
            _phases(*args, f"r{rep}_" if reps > 1 else "", phases)


def _phases(tc, nc, emb, emb_s, rp, rp_s, out_dram, embt, stript,
            W18, LTpad, identf, identb, ones18, r, which="ABCD"):
    # ---- Phase A: softmax stats (full batch -P/ne/1; strip L^T) ----
    # All Exp ops batch under one ACT table; the 72 Ln calls collapse into
    # ONE Ln over the collected sums (ACT table loads: ~2 instead of ~99).
    TT = BT + MT
    if "A" in which:
     with tc.tile_pool(name=f"{r}smx", bufs=1) as smx, \
          tc.tile_pool(name=f"{r}ltp", bufs=2, space="PSUM") as ltps:
        rp_sb = smx.tile([128, BT, E], F32, tag="rp_sb")
        rps_sb = smx.tile([128, MT, E], F32, tag="rps_sb")
        nc.sync.dma_start(
            out=rp_sb, in_=rp.rearrange("(bt p) e -> p bt e", p=128))
        nc.sync.dma_start(
            out=rps_sb, in_=rp_s.rearrange("(mt p) e -> p mt e", p=128))
        # Logits are N(0,1) so exp cannot overflow: skip the max-shift and
        # batch everything.  L = x - log(sum exp x);  ne = (sum e*x)/s - logs.
        e_all = smx.tile([128, TT, E], F32, tag="e_all")
        s_all = smx.tile([128, TT], F32, tag="s_all")
        logs_all = smx.tile([128, TT], F32, tag="logs_all")
        rs_all = smx.tile([128, TT], F32, tag="rs_all")
        nc.scalar.activation(out=e_all[:, 0:BT, :], in_=rp_sb, func=AF.Exp)
        nc.scalar.activation(out=e_all[:, BT:TT, :], in_=rps_sb, func=AF.Exp)
        nc.vector.reduce_sum(out=s_all, in_=e_all, axis=AX)
        nc.scalar.activation(out=logs_all, in_=s_all, func=AF.Ln)
        nc.vector.reciprocal(out=rs_all, in_=s_all)
        nc.vector.memset(W18[:, :, E + 1], 1.0)
        # ne_all = (sum_k e*x)*rs - logs, all [128, TT] batched
        prodel = smx.tile([128, TT, E], F32, tag="prodel")
        nc.vector.tensor_tensor(out=prodel[:, 0:BT, :], in0=e_all[:, 0:BT, :],
                                in1=rp_sb, op=OP.mult)
        nc.vector.tensor_tensor(out=prodel[:, BT:TT, :],
                                in0=e_all[:, BT:TT, :], in1=rps_sb,
                                op=OP.mult)
        epx = smx.tile([128, TT], F32, tag="epx")
        nc.vector.reduce_sum(out=epx, in_=prodel, axis=AX)
        ne_all = smx.tile([128, TT], F32, tag="ne_all")
        nc.vector.tensor_tensor(out=ne_all, in0=epx, in1=rs_all, op=OP.mult)
        nc.vector.tensor_tensor(out=ne_all, in0=ne_all, in1=logs_all,
                                op=OP.subtract)
        with nc.allow_low_precision(reason="ne copy to bf16 table"):
            nc.vector.tensor_copy(
                out=W18[:, :, E:E + 1],
                in_=ne_all.rearrange("p (t o) -> p t o", o=1)[:, 0:BT, :])
        for bt in range(BT):
            nc.vector.tensor_scalar(W18[:, bt, 0:E], e_all[:, bt, :],
                                    rs_all[:, bt:bt + 1], -1.0,
                                    op0=OP.mult, op1=OP.mult)
        # LTpad rows 0..15 = L^T, row 16 = 1 (via transpose of [L | 1]),
        # row 17 = 0 (whole-tile memset; partition slices must start at 0)
        nc.vector.memset(LTpad, 0.0)
        for ms in range(MT):
            t = BT + ms
            Lm = smx.tile([128, E + 1], F32, tag="Lm", bufs=3)
            nc.vector.memset(Lm[:, E:E + 1], 1.0)
            nc.vector.tensor_scalar(Lm[:, 0:E], rps_sb[:, ms, :],
                                    logs_all[:, t:t + 1], None,
                                    op0=OP.subtract)
            lt = ltps.tile([E + 1, 128], F32, tag="lt")
            nc.tensor.matmul(out=lt, lhsT=Lm, rhs=identf,
                             start=True, stop=True)
            nc.scalar.copy(out=LTpad[0:E + 1, ms * 128:(ms + 1) * 128],
                           in_=lt)

    # ---- Phase B+C: merged pipeline ----
    #   prep(bt):  DMA -> norm^2 (ACT) -> 16/||x|| (DVE) -> normalize (DVE)
    #              -> PE transpose -> fp8 copy-out (ACT/DVE alternating)
    #   simblk(jt): 8 DoubleRow matmuls (2 column halves) -> maskT (DVE)
    #              -> V matmul (accumulated over all jt)
    # simblk(jt) is emitted two prep steps behind, V one jt behind, so no
    # in-order engine stream ever waits on same-step work.
    do_b = "B" in which
    do_c = "C" in which
    if do_b or do_c:
     with tc.tile_pool(name=f"{r}vps", bufs=1, space="PSUM") as vps:
      V = vps.tile([W, STRIP], F32, name="V") if do_c else None
      with tc.tile_pool(name=f"{r}embp", bufs=3) as ep, \
           tc.tile_pool(name=f"{r}trps", bufs=2, space="PSUM") as trps, \
           tc.tile_pool(name=f"{r}simps", bufs=3, space="PSUM") as sps, \
           tc.tile_pool(name=f"{r}mkp", bufs=8) as mkp:
        tpend = []   # pending transpose copy-outs  (tp, dst, bt)
        vpend = []   # pending V matmuls            (jt, [msk0, msk1])

        def drain_tp():
            tp_, dst_, bt_ = tpend.pop(0)
            eng = (nc.scalar.copy if bt_ % 2 == 0 else nc.vector.tensor_copy)
            eng(out=dst_[:, :, bt_ * 128:(bt_ + 1) * 128],
                in_=tp_.rearrange("p (k c) -> p k c", k=KT))

        def prep(src_ap, dst_tile, bt):
            x = ep.tile([128, H], F32, tag="ex", bufs=6)
            # split loads across the SP hwdge queue and gpsimd's swdge queue
            # (gpsimd is otherwise idle, so its triggers never stall)
            dma_eng = nc.sync if bt % 2 == 0 else nc.gpsimd
            dma_eng.dma_start(out=x, in_=src_ap[bt * 128:(bt + 1) * 128, :])
            scr = ep.tile([128, H], BF16, tag="sqscr", bufs=2)
            ss = ep.tile([128, 1], F32, tag="ss", bufs=3)
            nc.scalar.activation(out=scr, in_=x, func=AF.Square,
                                 accum_out=ss)
            # n16 = ||x||/16;  rs = 16/||x||
            n16 = ep.tile([128, 1], F32, tag="n16", bufs=3)
            nc.scalar.activation(out=n16, in_=ss, func=AF.Sqrt, bias=0.0,
                                 scale=1.0 / (SCALE * SCALE))
            rs = ep.tile([128, 1], F32, tag="rs", bufs=3)
            nc.vector.reciprocal(out=rs, in_=n16)
            xq = ep.tile([128, H], BF16, tag="xq", bufs=3)
            nc.vector.tensor_scalar(xq, x, rs, None, op0=OP.mult)
            if tpend:
                drain_tp()
            tp = trps.tile([128, H], BF16, tag="tr")
            for kt in range(KT):
                nc.tensor.transpose(tp[:, kt * 128:(kt + 1) * 128],
                                    xq[:, kt * 128:(kt + 1) * 128],
                                    identb)
            tpend.append((tp, dst_tile, bt))

        def drain_v(stop):
            jt_, msks_ = vpend.pop(0)
            for hh in range(2):
                nc.tensor.matmul(out=V[:, hh * 512:(hh + 1) * 512],
                                 lhsT=W18[:, jt_, :], rhs=msks_[hh],
                                 start=(jt_ == 0), stop=stop)

        def simblk(jt):
            # k2 outer / hh inner: consecutive matmuls share the stationary
            # operand, so each LDWEIGHTS hides under 2x512 moving cycles
            simT0 = sps.tile([128, 512], F32, tag="simT0", bufs=2)
            simT1 = sps.tile([128, 512], F32, tag="simT1", bufs=2)
            halves = (simT0, simT1)
            for k2 in range(KT2):
                for hh in range(2):
                    nc.tensor.matmul(
                        out=halves[hh],
                        lhsT=embt[:, 2 * k2:2 * k2 + 2,
                                  jt * 128:(jt + 1) * 128],
                        rhs=stript[:, 2 * k2:2 * k2 + 2,
                                   hh * 512:(hh + 1) * 512],
                        start=(k2 == 0), stop=(k2 == KT2 - 1),
                        perf_mode=DR)
            msks = []
            for hh in range(2):
                msk = mkp.tile([128, 512], BF16, tag="mask")
                nc.vector.tensor_scalar(msk, halves[hh], THRESH, None,
                                        op0=OP.is_gt)
                msks.append(msk)
            if vpend:
                drain_v(False)
            vpend.append((jt, msks))

        if do_b:
            for ms in range(MT):           # strip prologue
                prep(emb_s, stript, ms)
        CLAG = 2
        for bt in range(BT):
            if do_b:
                prep(emb, embt, bt)
            if do_c and bt >= CLAG:
                simblk(bt - CLAG)
        for jt in range(BT - CLAG, BT) if do_c else []:
            simblk(jt)
        while tpend:
            drain_tp()
        if do_c:
            drain_v(True)

      # ---- readout: masked_sum and count from V (pipeline pools closed,
      # so the PSUM bank for the final matmul is free) ----
      if do_c:
            with tc.tile_pool(name=f"{r}fin", bufs=1) as fin, \
                 tc.tile_pool(name=f"{r}fps", bufs=1, space="PSUM") as fps:
                Vs = fin.tile([W, STRIP], F32)
                nc.scalar.copy(out=Vs, in_=V)
                scr = fin.tile([W, STRIP], F32)
                nc.vector.tensor_tensor(out=scr, in0=Vs, in1=LTpad,
                                        op=OP.mult)
                # accs col0 = rowsum(LTpad*V) (-> masked_sum), col1 =
                # rowsum(V); select row 17 of col1 (count) by multiplying
                # with [1 | e17] built from the identity's column 17.
                accs = fin.tile([W, 2], F32)
                nc.vector.reduce_sum(out=accs[:, 0:1], in_=scr, axis=AX)
                nc.vector.reduce_sum(out=accs[:, 1:2], in_=Vs, axis=AX)
                sel = fin.tile([W, 2], F32)
                nc.vector.tensor_copy(out=sel[:, 0:1], in_=ones18)
                nc.vector.tensor_copy(out=sel[:, 1:2],
                                      in_=identf[0:W, W - 1:W])
                msel = fin.tile([W, 2], F32)
                nc.vector.tensor_tensor(out=msel, in0=accs, in1=sel,
                                        op=OP.mult)
                res = fps.tile([1, 2], F32)
                nc.tensor.matmul(out=res, lhsT=ones18, rhs=msel,
                                 start=True, stop=True)
                out_sb = fin.tile([1, 2], F32)
                nc.scalar.copy(out=out_sb, in_=res)
                nc.sync.dma_start(out=out_dram, in_=out_sb)


def build_bass(reps=1, loop_iters=None, phases="ABCD"):
    nc = bacc.Bacc("TRN2", target_bir_lowering=False, debug=False)
    emb = nc.dram_tensor("emb", [B, H], F32, kind="ExternalInput").ap()
    emb_s = nc.dram_tensor("emb_strip", [STRIP, H], F32,
                           kind="ExternalInput").ap()
    rp = nc.dram_tensor("rp", [B, E], F32, kind="ExternalInput").ap()
    rp_s = nc.dram_tensor("rp_strip", [STRIP, E], F32,
                          kind="ExternalInput").ap()
    out = nc.dram_tensor("out", [1, 2], F32, kind="ExternalOutput").ap()
    with tile.TileContext(nc) as tc:
        _kernel(tc, emb, emb_s, rp, rp_s, out, reps=reps,
                loop_iters=loop_iters, phases=phases)
    nc.compile()
    return nc


_NC_CACHE = None


def make_in_maps(rp, emb):
    in_maps = []
    for d in range(NCORES):
        in_maps.append({
            "emb": emb,
            "emb_strip": np.ascontiguousarray(emb[d * STRIP:(d + 1) * STRIP]),
            "rp": rp,
            "rp_strip": np.ascontiguousarray(rp[d * STRIP:(d + 1) * STRIP]),
        })
    return in_maps


def kernel(routing_probs: np.ndarray, input_embeddings: np.ndarray,
           **_unused) -> np.ndarray:
    global _NC_CACHE
    if _NC_CACHE is None:
        _NC_CACHE = build_bass()
    nc = _NC_CACHE
    rp = np.ascontiguousarray(routing_probs, dtype=np.float32)
    emb = np.ascontiguousarray(input_embeddings, dtype=np.float32)
    in_maps = make_in_maps(rp, emb)
    res = run_bass_kernel_spmd(nc, in_maps, core_ids=list(range(NCORES)))
    vals = np.array([r["out"].reshape(2) for r in res.results],
                    dtype=np.float64)
    total = vals[:, 0].sum()
    cnt = vals[:, 1].sum() - B  # drop the diagonal pairs (kl there is 0)
    if cnt > 0:
        loss = np.float32(total) / np.float32(max(cnt, 1.0))
    else:
        loss = 0.0
    return np.array(WEIGHT * loss, dtype=np.float32)
